# revision 1
# baseline (speedup 1.0000x reference)
"""MoE FFN (top-1 routing) Trainium2 kernel — expert-parallel across 8 cores.

Strategy (per the expert-parallel sharding hint): the router gate and the
token dispatch ARE the sharding step, performed on the host inside kernel():
  - host computes router logits (x @ Wg + bg) and argmax expert ids
  - tokens are gathered per expert, padded to capacity C = max expert load
  - core e receives expert e's W1/W2/b1 plus its routed tokens, pre-tiled
    into contiguous-DMA layouts
  - the device runs the full FFN (both matmuls + exact gelu) in float32r
  - host scatters per-expert outputs back (adds b2 there, it is per-token
    constant) and un-shards to the full [B, S, D] output

Device kernel per core (C tokens, D=1024, H=4096):
  phase A: hT[m*128+p, c] = gelu(sum_k W1[k*128+p', m*128+p] x[c, k*128+p'])
           32 H-chunks x n_chunks x 8-matmul accumulation groups
  phase B: yT[d*128+p, c] = sum_k W2[k*128+p', d*128+p] hT[k*128+p', c]
           8 D-chunks x n_chunks x 32-matmul accumulation groups
Both matmuls consume the weights in their natural [K, M] orientation as the
stationary operand, so no transposes are needed anywhere on the device.
"""

import os
import sys

import numpy as np

for _p in ("/opt/trn_rl_repo", "/root/.axon_site/_ro/trn_rl_repo"):
    if os.path.isdir(_p) and _p not in sys.path:
        sys.path.insert(0, _p)

D_MODEL = 1024
D_HIDDEN = 4096
N_EXPERTS = 8
N_CORES = 8
P = 128
KD = D_MODEL // P  # 8 k-chunks over d_model
MH = D_HIDDEN // P  # 32 m-chunks over d_hidden

_compiled_cache = {}

# Set by the most recent kernel() call when BASS_KERNEL_TRACE=1: HW exec ns.
last_exec_time_ns = None
last_results = None


def _chunk_sizes(C):
    """Split C token columns into chunks <= 512, as evenly as possible.

    C >= 512 always (max expert load >= 4096/8), so chunks land in
    [256, 512] and float32r matmuls run at full 1 cycle/row speed.
    """
    nch = -(-C // 512)
    base, rem = divmod(C, nch)
    sizes = [base + 1] * rem + [base] * (nch - rem)
    return sizes


def _build_program(C):
    import concourse.mybir as mybir
    import concourse.tile as tile
    from concourse import bacc

    f32 = mybir.dt.float32
    f32r = mybir.dt.float32r

    nc = bacc.Bacc("TRN2", target_bir_lowering=False, debug=False,
                   num_devices=N_CORES)

    # Host-pretiled inputs (layouts chosen so each DMA is contiguous):
    #   xt  [128, KD*C]   xt[p, k*C + c]    = x[c, k*128+p]
    #   w1  [MH, 128, KD*128]  w1[m, p, k*128+j] = W1[k*128+j', m*128+... ]
    #        precisely: w1[m, p, k*128+j] = W1[k*128+p, m*128+j]  (lhsT tiles)
    #   w2  [KD, 128, MH*128]  w2[d, p, k*128+j] = W2[k*128+p, d*128+j]
    #   b1t [128, MH]     b1t[p, m] = b1[m*128+p]
    # Output:
    #   yt  [KD, 128, C]  yt[d, p, c] = y[c, d*128+p]  (pre-bias-b2)
    xt_d = nc.declare_dram_parameter("xt", [P, KD * C], f32r, isOutput=False)
    w1_d = nc.declare_dram_parameter("w1", [MH, P, KD * P], f32r, isOutput=False)
    w2_d = nc.declare_dram_parameter("w2", [KD, P, MH * P], f32r, isOutput=False)
    b1_d = nc.declare_dram_parameter("b1t", [P, MH], f32, isOutput=False)
    yt_d = nc.declare_dram_parameter("yt", [KD, P, C], f32, isOutput=True)

    chunks = _chunk_sizes(C)

    with tile.TileContext(nc) as tc:
        with (
            tc.tile_pool(name="persist", bufs=1) as persist,
            tc.tile_pool(name="w1p", bufs=4) as w1p,
            tc.tile_pool(name="w2p", bufs=3) as w2p,
            tc.tile_pool(name="outp", bufs=4) as outp,
            tc.tile_pool(name="psA", bufs=4, space="PSUM") as psA,
            tc.tile_pool(name="psB", bufs=4, space="PSUM") as psB,
        ):
            xt = persist.tile([P, KD * C], f32r)
            nc.sync.dma_start(out=xt[:], in_=xt_d[:])
            b1t = persist.tile([P, MH], f32)
            nc.sync.dma_start(out=b1t[:], in_=b1_d[:])
            ht = persist.tile([P, MH * C], f32r)

            # ---- Phase A: hT = gelu(W1^T x^T + b1) ----
            for m in range(MH):
                w1m = w1p.tile([P, KD * P], f32r, tag="w1m")
                nc.sync.dma_start(out=w1m[:], in_=w1_d[m])
                c0 = 0
                for cn in chunks:
                    ps = psA.tile([P, 512], mybir.dt.float32, tag="psA")
                    for k in range(KD):
                        nc.tensor.matmul(
                            ps[:, :cn],
                            w1m[:, k * P:(k + 1) * P],
                            xt[:, k * C + c0:k * C + c0 + cn],
                            start=(k == 0),
                            stop=(k == KD - 1),
                        )
                    nc.scalar.activation(
                        ht[:, m * C + c0:m * C + c0 + cn],
                        ps[:, :cn],
                        mybir.ActivationFunctionType.Gelu,
                        bias=b1t[:, m:m + 1],
                    )
                    c0 += cn

            # ---- Phase B: yT = W2^T hT ----
            for d in range(KD):
                w2d = w2p.tile([P, MH * P], f32r, tag="w2d")
                nc.sync.dma_start(out=w2d[:], in_=w2_d[d])
                c0 = 0
                for cn in chunks:
                    ps = psB.tile([P, 512], mybir.dt.float32, tag="psB")
                    for k in range(MH):
                        nc.tensor.matmul(
                            ps[:, :cn],
                            w2d[:, k * P:(k + 1) * P],
                            ht[:, k * C + c0:k * C + c0 + cn],
                            start=(k == 0),
                            stop=(k == MH - 1),
                        )
                    ot = outp.tile([P, 512], f32, tag="ot")
                    nc.vector.tensor_copy(ot[:, :cn], ps[:, :cn])
                    nc.sync.dma_start(
                        out=yt_d[d, :, c0:c0 + cn], in_=ot[:, :cn]
                    )
                    c0 += cn

    nc.compile()
    return nc


def _get_program(C):
    if C not in _compiled_cache:
        _compiled_cache[C] = _build_program(C)
    return _compiled_cache[C]


def kernel(x, Wg, bg, W1, b1, W2, b2):
    global last_exec_time_ns, last_results
    from concourse.bass_utils import run_bass_kernel_spmd

    x = np.asarray(x, dtype=np.float32)
    Wg = np.asarray(Wg, dtype=np.float32)
    bg = np.asarray(bg, dtype=np.float32)
    W1 = np.asarray(W1, dtype=np.float32)
    b1 = np.asarray(b1, dtype=np.float32)
    W2 = np.asarray(W2, dtype=np.float32)
    b2 = np.asarray(b2, dtype=np.float32)

    B, S, D = x.shape
    T = B * S
    xf = x.reshape(T, D)

    # ---- Router (replicated gate, computed host-side as the dispatch step)
    logits = xf @ Wg + bg
    eidx = np.argmax(logits, axis=-1)

    tok = [np.nonzero(eidx == e)[0] for e in range(N_EXPERTS)]
    counts = [len(t) for t in tok]
    C = max(max(counts), 512)
    C = ((C + 7) // 8) * 8  # mild alignment for DMA friendliness

    nc = _get_program(C)

    # ---- Build per-core pre-tiled inputs
    in_maps = []
    for e in range(N_EXPERTS):
        n_e = counts[e]
        xe = xf[tok[e]]  # [n_e, D]
        xt = np.zeros((P, KD * C), dtype=np.float32)
        xeT = np.ascontiguousarray(xe.T).reshape(KD, P, n_e)
        for k in range(KD):
            xt[:, k * C:k * C + n_e] = xeT[k]
        # lhsT tiles, contiguous per m-chunk / d-chunk
        w1 = np.ascontiguousarray(
            W1[e].reshape(KD, P, MH, P).transpose(2, 1, 0, 3).reshape(MH, P, KD * P)
        )
        w2 = np.ascontiguousarray(
            W2[e].reshape(MH, P, KD, P).transpose(2, 1, 0, 3).reshape(KD, P, MH * P)
        )
        b1t = np.ascontiguousarray(b1[e].reshape(MH, P).T)
        in_maps.append({"xt": xt, "w1": w1, "w2": w2, "b1t": b1t})

    trace = os.environ.get("BASS_KERNEL_TRACE", "") == "1"
    if trace:
        try:
            import axon_profile_shim

            axon_profile_shim.install()
        except ImportError:
            pass

    res = run_bass_kernel_spmd(nc, in_maps, list(range(N_CORES)), trace=trace)
    last_exec_time_ns = res.exec_time_ns
    last_results = res

    # ---- Combine: scatter tokens back, add b2 host-side
    out = np.zeros((T, D), dtype=np.float32)
    for e in range(N_EXPERTS):
        n_e = counts[e]
        if n_e == 0:
            continue
        yt = res.results[e]["yt"]  # [KD, P, C]
        ye = yt.reshape(D, C)[:, :n_e].T  # [n_e, D]
        out[tok[e]] = ye + b2[e][None, :]
    return out.reshape(B, S, D)
